# revision 12
# baseline (speedup 1.0000x reference)
"""Sparse 2x2x2/stride-2 "generative" convolution on Trainium2 (8 NeuronCores).

Reference semantics: inputs are N=300000 occupied voxels of a 128^3 grid with
32-ch features.  Each output cell is an even 2x2x2 block origin containing
>=1 input voxel; its feature is

    out[m] = sum_{k in 0..7, valid} features[idx(m, k)] @ W[k]      (W: [8,32,64])

where idx(m,k) is the input row at corner k of block m.

Host (numpy): index math only — block ids, sorted-unique output list, the
[M, 8] gather table, sharding (positions are compile-time constants, exactly
as in the reference).  Device (Bass/Tile): all data movement + compute —
indirect-DMA row gathers (TRN2 semantics: one int32 offset per partition,
CIN contiguous elements each; invalid corners read a zero row), PE transpose
to put channels on partitions, PE matmul against the stacked [256, 64]
weight accumulating in PSUM.

Output sharding: core c produces rows [c*Mpc, (c+1)*Mpc) of the M sorted
output blocks, laid out on device as [128, T*64] (partition-major tiles) so
the final store is one contiguous DMA; host restores row order.
"""

import numpy as np

import concourse.bass as bass
import concourse.bacc as bacc
import concourse.mybir as mybir
from concourse.tile import TileContext
from concourse.masks import make_identity
from concourse.bass_utils import run_bass_kernel_spmd

NCORES = 8
P = 128
CIN = 32
COUT = 64
B = 64              # output block grid extent (128/2)


def _build_program(N, T):
    """One SPMD program, identical on all 8 cores.  T = output tiles per core,
    each tile = 128 output rows."""
    f32 = mybir.dt.float32
    i32 = mybir.dt.int32
    nc = bacc.Bacc(None, target_bir_lowering=False)

    feat = nc.declare_dram_parameter("features", [N + 1, CIN], f32, isOutput=False)
    idxd = nc.declare_dram_parameter("idx", [P, T * 8], i32, isOutput=False)
    wpd = nc.declare_dram_parameter("wp", [P, 2 * COUT], f32, isOutput=False)
    outd = nc.declare_dram_parameter("out", [P, T * COUT], f32, isOutput=True)

    with TileContext(nc) as tc:
        with (
            tc.tile_pool(name="const", bufs=1) as cpool,
            tc.tile_pool(name="gbuf", bufs=4) as gpool,
            tc.tile_pool(name="gtbuf", bufs=3) as gtpool,
            tc.tile_pool(name="ptp", bufs=2, space="PSUM") as ptpool,
            tc.tile_pool(name="pop", bufs=2, space="PSUM") as popool,
        ):
            ident = cpool.tile([P, P], f32)
            make_identity(nc, ident[:, :])
            wt = cpool.tile([P, 2 * COUT], f32)
            nc.sync.dma_start(out=wt[:, :], in_=wpd[:, :])
            idx_sb = cpool.tile([P, T * 8], i32)
            nc.sync.dma_start(out=idx_sb[:, :], in_=idxd[:, :])
            out_sb = cpool.tile([P, T * COUT], f32)

            for t in range(T):
                # 8 indirect gathers: one int32 row offset per partition,
                # CIN contiguous f32 each -> G[:, k*32:(k+1)*32]
                Gt = gpool.tile([P, 8 * CIN], f32, tag="G")
                for k in range(8):
                    nc.gpsimd.indirect_dma_start(
                        out=Gt[:, k * CIN : (k + 1) * CIN],
                        out_offset=None,
                        in_=feat[:, :],
                        in_offset=bass.IndirectOffsetOnAxis(
                            ap=idx_sb[:, t * 8 + k : t * 8 + k + 1], axis=0
                        ),
                    )
                pt = ptpool.tile([P, 2 * P], f32, tag="pt")
                nc.tensor.transpose(
                    out=pt[:, 0:P], in_=Gt[:, 0:P], identity=ident[:, :]
                )
                nc.tensor.transpose(
                    out=pt[:, P : 2 * P], in_=Gt[:, P : 2 * P], identity=ident[:, :]
                )
                gt = gtpool.tile([P, 2 * P], f32, tag="gt")
                nc.vector.tensor_copy(out=gt[:, :], in_=pt[:, :])
                po = popool.tile([P, COUT], f32, tag="po")
                nc.tensor.matmul(
                    out=po[:, :], lhsT=gt[:, 0:P], rhs=wt[:, 0:COUT],
                    start=True, stop=False,
                )
                nc.tensor.matmul(
                    out=po[:, :], lhsT=gt[:, P : 2 * P], rhs=wt[:, COUT : 2 * COUT],
                    start=False, stop=True,
                )
                nc.vector.tensor_copy(
                    out=out_sb[:, t * COUT : (t + 1) * COUT], in_=po[:, :]
                )
            nc.sync.dma_start(out=outd[:, :], in_=out_sb[:, :])
    nc.compile()
    return nc


def kernel(features, in_positions, weight):
    feats = np.ascontiguousarray(np.asarray(features, dtype=np.float32))
    pos = np.asarray(in_positions).astype(np.int64)
    w = np.asarray(weight, dtype=np.float32)
    N = feats.shape[0]

    # ---- host index preprocessing (positions are constants in the reference) ----
    block = pos >> 1
    kid = ((pos[:, 0] & 1) << 2) | ((pos[:, 1] & 1) << 1) | (pos[:, 2] & 1)
    blin = (block[:, 0] * B + block[:, 1]) * B + block[:, 2]
    uniq, inv = np.unique(blin, return_inverse=True)
    M = uniq.shape[0]

    out_int = np.stack([uniq // (B * B), (uniq // B) % B, uniq % B], axis=1) * 2
    out_positions = ((out_int.astype(np.float32) + 0.5) / 2.0).astype(np.float32)

    # per-core padded tiling
    Mpc = -(-M // NCORES)                  # rows per core before padding
    T = -(-Mpc // P)                       # 128-row tiles per core
    Mpc = T * P

    INVALID = N                            # all-zero row of the feature table
    idx8 = np.full((NCORES * Mpc, 8), INVALID, dtype=np.int32)
    idx8[inv, kid] = np.arange(N, dtype=np.int32)

    feats_dev = np.vstack([feats, np.zeros((1, CIN), np.float32)])
    wp = w.reshape(8 * CIN, COUT)
    wp_dev = np.ascontiguousarray(np.concatenate([wp[:P], wp[P:]], axis=1))
    idx_dev = [
        np.ascontiguousarray(
            idx8[c * Mpc : (c + 1) * Mpc]
            .reshape(T, P, 8)
            .transpose(1, 0, 2)
            .reshape(P, T * 8)
        )
        for c in range(NCORES)
    ]

    nc = _build_program(N, T)
    in_maps = [
        {"features": feats_dev, "idx": idx_dev[c], "wp": wp_dev}
        for c in range(NCORES)
    ]
    res = run_bass_kernel_spmd(nc, in_maps, list(range(NCORES)))
    global LAST_RESULTS
    LAST_RESULTS = res

    parts = [
        res.results[c]["out"].reshape(P, T, COUT).transpose(1, 0, 2).reshape(Mpc, COUT)
        for c in range(NCORES)
    ]
    out_feat = np.concatenate(parts, axis=0)[:M].astype(np.float32)
    return out_feat, out_positions
